# revision 6
# baseline (speedup 1.0000x reference)
"""Blocksparse conv2d (3x3, stride 1, pad 1) on 8 Trainium2 NeuronCores.

Strategy
--------
Data-parallel over batch: 16 images -> 2 per core, identical SPMD program.

The mask zeroes whole 32x32 (cout, cin) channel blocks; the host inspects
the runtime mask and specializes the schedule on BOTH channel axes:
 - only surviving input-channel blocks are shipped/loaded (K_used chans);
 - only surviving output-channel blocks are computed/stored (OL chans).
   Dead out-channels are exactly bias (their weights are exactly zero), so
   the host broadcasts bias into them for free -- the device neither
   computes nor stores them.

When 2*K_used <= 128 the two images of a core are PAIRED across PE row
groups (img0 on partitions 0:K_used, img1 on K_used:2*K_used) so their
matmul streams run concurrently on independent row tiles of the systolic
array -- full-array throughput without duplicating any x data.

Conv is lowered to 9 shifted matmuls accumulating in PSUM.  The host
pre-pads each image with a zero border (130x130), so every tap is a clean
2D-strided view of one flat SBUF buffer.  The x load is chunked into 12
row-band HWDGE DMAs so the matmul pipeline starts as soon as the first
small band lands.  Tap-0 weights ship as their own small DMA ahead of the
rest so the first LDWEIGHTS isn't gated on the full weight tensor.

x and the host-side masked/transposed weights are bf16 (host-cast); the
y path is ALSO bf16: the PSUM->SBUF bias-add copy casts f32->bf16 and the
stores ship bf16 (host casts back to f32).  This halves store traffic
(which removed ~8us of stage-recycle stalls + final drain in the f32
version) and costs ~1e-3 extra relative error against a 2e-2 gate.

The PE clock is HAM-gated: cold = 1.2 GHz, warming to 2.4 GHz only after
~3.4us of sustained matmul activity.  The first ~12us of the kernel are
DMA latency (preamble + DGE descriptor latency + first transfers), so a
chain of dummy warm-up matmuls on zeroed scratch tiles runs during that
window, releasing the throttle before the first real matmul issues
(measured ~2.4us saved: the first ~25 real matmul pairs otherwise run at
~430ns cadence instead of 218ns).

Windows are processed one pair at a time (one PSUM bank per image), so the
8-bank pool holds 4 pairs in flight: steady state is the PE's 218ns/pair
streaming limit with LDWEIGHTS hidden by the pull-ahead window.  Bias is
fused into the PSUM->SBUF copy, alternating scalar/vector engines; y
stores alternate the two HWDGE rings with tapered final chunks.
"""

import ml_dtypes
import numpy as np
from contextlib import ExitStack

import concourse.bass as bass
import concourse.tile as tile
from concourse import mybir, bacc
from concourse import bass_utils
# Problem shape (hardcoded per contract)
B, CIN, COUT, H, W = 16, 128, 128, 128, 128
KH, KW = 3, 3
BLK = 32
NCORES = 8
BPC = B // NCORES            # images per core
PH, PW = H + 2, W + 2        # host zero-padded image (130 x 130)
FLAT = PH * PW

RPW = 4                      # output rows per PSUM window (N = 512 = full bank)
NWIN = H // RPW              # 32 windows
CHUNKS = [8, 8, 8, 4, 2, 1, 1]  # windows per output-DMA chunk (tapered tail: the
                             # last chunks are tiny so the final copy->store->
                             # completion chain after the last matmul is short)
GROUPS = [[1] * c for c in CHUNKS]

# Dummy matmuls to release the HAM clock gate.  Measured: warmup starts
# ~6.1us (right after the vector-engine memsets clear the preamble), the
# first x chunk lands ~10.0us, and each cold dummy takes ~427ns -- 10
# dummies bridge the PE-busy window past the data-ready point with ~0.4us
# of margin and no over-run (the engine queue is in-order, so excess
# warmup directly delays the first real matmul).

_cache = {}
_last_in_maps = None


def _build(n_ib, n_ob, paired):
    """Build + compile the per-core SPMD program.

    n_ib:   number of surviving 32-channel input blocks (1..4)
    n_ob:   number of surviving 32-channel output blocks (1..4)
    paired: both images share the partition dim on separate PE row groups
    """
    K_used = BLK * n_ib
    OL = BLK * n_ob
    reps = 2 if paired else 1
    DK = reps * K_used
    assert DK <= 128

    nc = bacc.Bacc("TRN2", target_bir_lowering=False, debug=False)
    f32 = mybir.dt.float32
    bf16 = mybir.dt.bfloat16

    x_in = nc.dram_tensor("x", [BPC, K_used, PH, PW], bf16, kind="ExternalInput").ap()
    w_in = nc.dram_tensor("wt", [DK, KH * KW, OL], bf16, kind="ExternalInput").ap()
    b_in = nc.dram_tensor("bias", [OL], f32, kind="ExternalInput").ap()
    y_out = nc.dram_tensor("y", [BPC, OL, H, W], bf16, kind="ExternalOutput").ap()

    # x-load chunk boundaries (padded-image rows).  Window w reads padded rows
    # 4w..4w+5, so a boundary at 4k+6 releases windows 0..k.  The first
    # boundary at 4 releases just window 0's dh=0 taps ~0.25us earlier (the
    # head is DGE-latency dominated, so the first transfer should be minimal).
    bounds = [0, 4, 6, 14, 22, 30]
    while bounds[-1] + 16 < PH:
        bounds.append(bounds[-1] + 16)
    bounds.append(PH)

    with tile.TileContext(nc) as tc:
        with ExitStack() as ctx:
            singles = ctx.enter_context(tc.tile_pool(name="singles", bufs=1))
            stage_pool = ctx.enter_context(tc.tile_pool(name="ystage", bufs=4))
            psum_pool = ctx.enter_context(
                tc.tile_pool(name="psum", bufs=8, space="PSUM")
            )

            # Weights + bias on the Activation HWDGE ring so they drain in
            # parallel with the x chunks on the SP ring.  Tap-0 weights are a
            # separate small DMA so the first LDWEIGHTS is gated on ~24KB,
            # not the whole weight tensor.
            wT = singles.tile([DK, KH * KW, OL], bf16, name="wT2")
            nc.scalar.dma_start(out=wT[:, 0:1, :], in_=w_in[:, 0:1, :])
            nc.scalar.dma_start(out=wT[:, 1:, :], in_=w_in[:, 1:, :])
            bias_sb = singles.tile([OL, 1], f32, name="bias_sb")
            nc.scalar.dma_start(out=bias_sb, in_=b_in.unsqueeze(1))

            def load_chunks(xbt, src):
                for k in range(len(bounds) - 1):
                    lo, hi = bounds[k] * PW, bounds[k + 1] * PW
                    nc.sync.dma_start(out=xbt[:, lo:hi], in_=src[:, lo:hi])

            if paired:
                xb = singles.tile([DK, FLAT], bf16, name="xb")
                load_chunks(xb, x_in.rearrange("b c h w -> (b c) (h w)"))
                xbufs = [xb] * BPC
                img_base = [i * K_used for i in range(BPC)]
            else:
                xbufs, img_base = [], []
                for b in range(BPC):
                    xbi = singles.tile([K_used, FLAT], bf16, name=f"xb{b}")
                    load_chunks(xbi, x_in[b].rearrange("c h w -> c (h w)"))
                    xbufs.append(xbi)
                    img_base.append(0)

            # --- PE warm-up: release the HAM clock gate during the DMA head.
            # Zeroed scratch tiles -> a chain of dummy matmuls keeps the PE
            # busy from the end of the framework preamble (~6us) until the
            # first real matmul's data lands (~11us), so real matmuls start
            # at 2.4GHz instead of ramping from 1.2GHz.
            N_WARMUP = 6
            wdum = singles.tile([128, 128], bf16, name="wdum")
            xdum = singles.tile([128, 512], bf16, name="xdum")
            nc.vector.memset(wdum[:], 0.0)
            nc.vector.memset(xdum[:], 0.0)
            ps_dum = psum_pool.tile([128, 512], f32, tag="ps", name="ps_warm")
            for i in range(N_WARMUP):
                nc.tensor.matmul(
                    ps_dum, wdum, xdum,
                    start=(i == 0), stop=(i == N_WARMUP - 1),
                )

            assert sum(CHUNKS) == NWIN
            assert [sum(g) for g in GROUPS] == CHUNKS
            c0 = 0
            for nwc, chunk_groups in zip(CHUNKS, GROUPS):
                wins = list(range(c0, c0 + nwc))
                chunk_r0 = RPW * c0
                chunk_nr = RPW * len(wins)
                c0 += nwc
                stages = [
                    stage_pool.tile(
                        [OL, RPW * max(CHUNKS), W], bf16, tag="stage",
                        name=f"st{b}_{chunk_r0}",
                    )
                    for b in range(BPC)
                ]
                g0 = 0
                for gsz in chunk_groups:
                    group = wins[g0 : g0 + gsz]
                    g0 += gsz
                    ps = {}
                    for b in range(BPC):
                        for w in group:
                            ps[(b, w)] = psum_pool.tile(
                                [OL, 512], f32, tag="ps", name=f"ps{b}_{w}"
                            )
                    # tap-outer, image-inner: the two images' row groups
                    # alternate so each group's LDWEIGHTS hides behind the
                    # other group's matmuls.
                    for t in range(KH * KW):
                        dh, dw = divmod(t, KW)
                        for b in range(BPC):
                            base = img_base[b]
                            xbi = xbufs[b]
                            lhsT = wT[base : base + K_used, t, :]
                            for w in group:
                                q0 = (RPW * w + dh) * PW + dw
                                v = xbi[base : base + K_used, q0 : q0 + 1]
                                rhs = bass.AP(
                                    tensor=v.tensor,
                                    offset=v.offset,
                                    ap=[list(v.ap[0]), [PW, RPW], [1, W]],
                                )
                                nc.tensor.matmul(
                                    ps[(b, w)][:, : RPW * W],
                                    lhsT,
                                    rhs,
                                    start=(t == 0),
                                    stop=(t == KH * KW - 1),
                                )
                    # copy-out with fused bias + f32->bf16 cast, split across
                    # ACT and DVE
                    for b in range(BPC):
                        for w in group:
                            r0 = RPW * w
                            ps_v = ps[(b, w)][:, : RPW * W].rearrange(
                                "p (r s) -> p r s", s=W
                            )
                            dst = stages[b][:, r0 - chunk_r0 : r0 - chunk_r0 + RPW, :]
                            if (w + b) % 2 == 0:
                                nc.scalar.activation(
                                    out=dst,
                                    in_=ps_v,
                                    func=mybir.ActivationFunctionType.Identity,
                                    bias=bias_sb,
                                    scale=1.0,
                                )
                            else:
                                nc.vector.tensor_scalar_add(
                                    out=dst, in0=ps_v, scalar1=bias_sb
                                )
                # alternate the two HWDGE rings (SP / Activation) so the small
                # final stores don't queue behind every earlier large store
                for b in range(BPC):
                    eng = nc.sync if b == 0 else nc.scalar
                    eng.dma_start(
                        out=y_out[b][:, chunk_r0 : chunk_r0 + chunk_nr, :],
                        in_=stages[b][:, :chunk_nr, :],
                    )

    nc.compile()
    return nc


def kernel(x, weight, bias, mask):
    x = np.ascontiguousarray(np.asarray(x, dtype=np.float32))
    weight = np.asarray(weight, dtype=np.float32)
    bias = np.ascontiguousarray(np.asarray(bias, dtype=np.float32))
    mask = np.asarray(mask, dtype=np.float32)

    # --- host-side schedule specialization from the runtime mask ----------
    wm = weight * mask
    blk_any_i = (
        np.abs(wm).reshape(COUT, CIN // BLK, BLK, KH, KW).sum(axis=(0, 2, 3, 4)) > 0
    )
    used_ibs = [ib for ib in range(CIN // BLK) if blk_any_i[ib]] or [0]
    n_ib = len(used_ibs)
    K_used = BLK * n_ib
    blk_any_o = (
        np.abs(wm).reshape(COUT // BLK, BLK, CIN, KH, KW).sum(axis=(1, 2, 3, 4)) > 0
    )
    used_obs = [ob for ob in range(COUT // BLK) if blk_any_o[ob]] or [0]
    n_ob = len(used_obs)
    OL = BLK * n_ob
    paired = (BPC == 2) and (2 * K_used <= 128)
    reps = 2 if paired else 1

    used_ch = np.concatenate(
        [np.arange(ib * BLK, (ib + 1) * BLK) for ib in used_ibs]
    )
    live_ch = np.concatenate(
        [np.arange(ob * BLK, (ob + 1) * BLK) for ob in used_obs]
    )
    dead_ch = np.setdiff1d(np.arange(COUT), live_ch)

    key = (n_ib, n_ob, paired)
    if key not in _cache:
        _cache[key] = _build(n_ib, n_ob, paired)
    nc = _cache[key]

    # wT[rep*K_used + c, t, o] = (w*m)[live_o, used_ch[c], tap t], bf16
    wT = wm[np.ix_(live_ch, used_ch)].reshape(OL, K_used, KH * KW).transpose(1, 2, 0)
    wT = np.ascontiguousarray(
        np.concatenate([wT] * reps, axis=0).astype(ml_dtypes.bfloat16)
    )

    # zero-padded x (130x130) restricted to the used channels, host-cast bf16
    xp = np.zeros((B, K_used, PH, PW), dtype=ml_dtypes.bfloat16)
    xp[:, :, 1 : H + 1, 1 : W + 1] = x[:, used_ch].astype(ml_dtypes.bfloat16)

    bias_live = np.ascontiguousarray(bias[live_ch])

    in_maps = []
    for core in range(NCORES):
        xs = np.ascontiguousarray(xp[core * BPC : (core + 1) * BPC])
        in_maps.append({"x": xs, "wt": wT, "bias": bias_live})

    global _last_in_maps
    _last_in_maps = in_maps

    res = bass_utils.run_bass_kernel_spmd(nc, in_maps, core_ids=list(range(NCORES)))
    ybf = np.concatenate([res.results[c]["y"] for c in range(NCORES)], axis=0)

    y = np.empty((B, COUT, H, W), dtype=np.float32)
    y[:, live_ch] = ybf.astype(np.float32)
    if dead_ch.size:
        y[:, dead_ch] = bias[dead_ch][None, :, None, None]
    return y
